# revision 1
# baseline (speedup 1.0000x reference)
"""GCNConvSC (residual + GCNConv) Trainium2 Bass kernel, 8-core SPMD.

Math (matches the PyG-style reference):
    deg[v]  = indeg_with_selfloop(v)          (count of v in dst, +1)
    u       = deg^{-1/2}
    y       = u[:,None] * x                   (pre-scaled node features)
    z[v]    = sum_{e: dst_e = v} y[src_e]     (unweighted edge aggregation)
    out[v]  = x[v] + b + (u[v] * (z[v] + y[v])) @ W

The per-edge norm u[src]*u[dst] factorizes: u[src] folds into y (gather
source), u[dst] is a post-aggregation row scale, and the self-loop term
u[v]^2*x[v] is the acc's ys initialization. The matmul by W commutes with
the segment-sum, so it runs once per node after aggregation.

Sharding: destination nodes are range-partitioned over the 8 cores
(12544 dst slots per core). Each core gathers y[src] rows for its edges
from a replicated y in its HBM via dma_gather (int16 indices => 4 source
chunks of 25024 rows), and aggregates them with one-hot matmuls on the
tensor engine into PSUM windows of 128 dst slots (feat-major), 4 windows
per PSUM bank. The one-hot [128 edges x 128 slots] for each edge tile is
built on the vector engine as (iota == slot) with a staged iota tile.
Edges are sorted by (window-group, src-chunk, window) on the host and
padded per (chunk, window) run to multiples of 128 so every matmul is
window-pure; pad edges use src index 0 with slot -1 (one-hot row = 0).

The schedule (tile counts per (group, chunk, window)) is shared across
all 8 cores (SPMD single program), using the max count over cores.
"""

import sys

sys.path.insert(0, "/opt/trn_rl_repo")

import numpy as np

N_NODES = 100000
F = 128
N_CORES = 8
S = 12544            # dst slots per core (98 windows of 128)
WN = 98              # windows per core
WG_SIZE = 4          # windows per PSUM bank group
N_CHUNKS = 4
CHUNK = 25024        # gather-source rows per chunk (int16-safe)
NPAD = N_CHUNKS * CHUNK  # 100096 padded node rows for y

import os
MSGS_DT = os.environ.get("GCN_MSGS_DT", "bfloat16")  # gathered messages (y), matmul lhsT
OH_DT = os.environ.get("GCN_OH_DT", "bfloat16")      # iota/slots/one-hot (matmul rhs)


def _host_plan(edge_index):
    """Sort/bucket edges per core; emit the shared SPMD schedule plus
    per-core gather-index and slot arrays."""
    src = np.asarray(edge_index[0], dtype=np.int64)
    dst = np.asarray(edge_index[1], dtype=np.int64)

    deg_e = np.bincount(dst, minlength=N_NODES)
    u = (1.0 / np.sqrt(deg_e.astype(np.float64) + 1.0)).astype(np.float32)

    # Deal dsts snake-wise by descending degree across cores: every core's
    # position-p dst has ~the same degree, so per-(chunk, window) counts are
    # nearly equal across cores and the shared max-based schedule pads little.
    order = np.argsort(-deg_e, kind="stable")
    i = np.arange(N_NODES)
    blk, lane = i // N_CORES, i % N_CORES
    core_i = np.where(blk % 2 == 0, lane, N_CORES - 1 - lane)
    # perm[c, p] = global dst at (core c, slot position p); -1 = empty slot
    perm = np.full((N_CORES, S), -1, dtype=np.int64)
    perm[core_i, blk] = order
    core_of_node = np.empty(N_NODES, dtype=np.int64)
    pos_of_node = np.empty(N_NODES, dtype=np.int64)
    core_of_node[order] = core_i
    pos_of_node[order] = blk

    core_of = core_of_node[dst]
    pos_e_all = pos_of_node[dst]
    u_e_all = u[dst]
    chunk_of = src // CHUNK

    # per-core, per-(window, chunk) edge lists
    per_core = []
    counts = np.zeros((N_CORES, N_CHUNKS, WN), dtype=np.int64)
    for c in range(N_CORES):
        m = core_of == c
        es, pos_e, ue = src[m], pos_e_all[m], u_e_all[m]
        ch = chunk_of[m]
        w = pos_e // 128
        slot = pos_e % 128
        # sort edges by (window-group, chunk, window)
        wg = w // WG_SIZE
        so = np.lexsort((w, ch, wg))
        es, slot, ch, w, ue = es[so], slot[so], ch[so], w[so], ue[so]
        np.add.at(counts[c], (ch, w), 1)
        per_core.append((es, slot, ch, w, ue))

    # shared schedule: tiles per (chunk, window) = max over cores
    n_tiles = np.maximum((counts.max(axis=0) + 127) // 128, 0)  # [N_CHUNKS, WN]
    # every window needs >=1 tile overall so its PSUM quarter gets written
    empty_w = n_tiles.sum(axis=0) == 0
    n_tiles[0, empty_w] = 1

    # global tile order: for wg, for chunk, for window in wg
    n_wg = (WN + WG_SIZE - 1) // WG_SIZE
    sched = []  # list of segments: (chunk, [(window, q, ntiles, first, last)])
    T = 0
    for g in range(n_wg):
        ws = range(g * WG_SIZE, min((g + 1) * WG_SIZE, WN))
        touched = [w for w in ws if n_tiles[:, w].sum() > 0]
        first_touch = {w: None for w in touched}
        last_touch = {w: None for w in touched}
        segs = []
        for ch in range(N_CHUNKS):
            tl = []
            for w in ws:
                nt = int(n_tiles[ch, w])
                if nt == 0:
                    continue
                tl.append([w, w % WG_SIZE, nt])
                if first_touch[w] is None:
                    first_touch[w] = (ch, w)
                last_touch[w] = (ch, w)
            segs.append(tl)
        sched.append((g, segs, first_touch, last_touch))
        T += int(n_tiles[:, list(ws)].sum())

    # per-core padded edge streams in schedule order
    idx16 = np.zeros((N_CORES, T * 128), dtype=np.int16)
    slots = np.full((N_CORES, T * 128), -1.0, dtype=np.float32)
    uvals = np.zeros((N_CORES, T * 128), dtype=np.float32)
    for c in range(N_CORES):
        es, eslot, ch, w, ue = per_core[c]
        # edges are sorted by (wg, chunk, window); walk in the same order
        keys = list(zip(w // WG_SIZE, ch, w))
        run_start = {}
        for i, k in enumerate(keys):
            if k not in run_start:
                run_start[k] = i
        run_len = counts[c]
        out_pos = 0
        for g, segs, _, _ in sched:
            for chp in range(N_CHUNKS):
                for wseg, q, nt in segs[chp]:
                    cnt = int(run_len[chp, wseg])
                    if cnt > 0:
                        i0 = run_start[(g, chp, wseg)]
                        sl = slice(i0, i0 + cnt)
                        local = (es[sl] - chp * CHUNK).astype(np.int16)
                        idx16[c, out_pos : out_pos + cnt] = local
                        slots[c, out_pos : out_pos + cnt] = eslot[sl].astype(
                            np.float32
                        )
                        uvals[c, out_pos : out_pos + cnt] = ue[sl].astype(np.float32)
                    out_pos += nt * 128
        assert out_pos == T * 128

    return u, n_tiles, sched, T, idx16, slots, uvals, perm


def _build_program(T, sched, repeat=1):
    import concourse.bacc as bacc
    import concourse.mybir as mybir
    from concourse import tile

    dt = getattr(mybir.dt, MSGS_DT)
    oh_dt = getattr(mybir.dt, OH_DT)
    f32 = mybir.dt.float32

    nc = bacc.Bacc(
        "TRN2",
        target_bir_lowering=False,
        debug=False,
        enable_asserts=True,
        num_devices=N_CORES,
    )

    y_d = nc.dram_tensor("y", [NPAD, F], dt, kind="ExternalInput").ap()
    idx_d = nc.dram_tensor("idx16", [128, T * 8], mybir.dt.int16, kind="ExternalInput").ap()
    slots_d = nc.dram_tensor("slots", [128, T], f32, kind="ExternalInput").ap()
    uvals_d = nc.dram_tensor("uvals", [128, T], f32, kind="ExternalInput").ap()
    iota_d = nc.dram_tensor("iota", [128, 128], f32, kind="ExternalInput").ap()
    ysT_d = nc.dram_tensor("ysT", [128, S], f32, kind="ExternalInput").ap()
    xsT_d = nc.dram_tensor("xsT", [128, S], f32, kind="ExternalInput").ap()
    w_d = nc.dram_tensor("W", [F, F], f32, kind="ExternalInput").ap()
    out_d = nc.dram_tensor("outT", [128, S], f32, kind="ExternalOutput").ap()

    with tile.TileContext(nc) as tc:
        with (
            tc.tile_pool(name="const", bufs=1) as const_p,
            tc.tile_pool(name="acc", bufs=1) as acc_p,
            tc.tile_pool(name="msgs", bufs=4) as msgs_p,
            tc.tile_pool(name="oh", bufs=8) as oh_p,
            tc.tile_pool(name="psum", bufs=6, space="PSUM") as psum_p,
            tc.tile_pool(name="fin", bufs=2) as fin_p,
            tc.tile_pool(name="fpsum", bufs=2, space="PSUM") as fpsum_p,
        ):
            idx_sb = const_p.tile([128, T * 8], mybir.dt.int16)
            slots_sb = const_p.tile([128, T], f32)
            uvals_sb = const_p.tile([128, T], f32)
            iota_sb = const_p.tile([128, 128], f32)
            w_sb = const_p.tile([F, F], f32)
            acc = acc_p.tile([128, S], f32)

            nc.sync.dma_start(idx_sb[:], idx_d[:])
            nc.sync.dma_start(slots_sb[:], slots_d[:])
            nc.sync.dma_start(uvals_sb[:], uvals_d[:])
            nc.sync.dma_start(iota_sb[:], iota_d[:])
            nc.sync.dma_start(w_sb[:], w_d[:])

            # repeat>1 is a benchmarking mode: re-runs the whole body so
            # per-dispatch tunnel overhead cancels in wall-time differences
            for _rep in range(repeat):
                # acc starts as ys^T (self-loop term y[v], scaled later by u[v])
                nc.sync.dma_start(acc[:], ysT_d[:])

                g_tile = 0  # global tile cursor
                for g, segs, first_touch, last_touch in sched:
                    # one PSUM bank per window in this group
                    psums = {w: psum_p.tile([128, 128], f32, tag="psum", name=f"ps_w{w}")
                             for w in first_touch}
                    for ch in range(N_CHUNKS):
                        seg_tiles = sum(nt for (_, _, nt) in segs[ch])
                        if seg_tiles == 0:
                            continue
                        n_idx = seg_tiles * 128
                        msgs = msgs_p.tile([128, seg_tiles * 128], dt, tag="msgs")
                        m3 = msgs[:].rearrange("p (b f) -> p b f", f=F)
                        nc.gpsimd.dma_gather(
                            m3,
                            y_d[ch * CHUNK : (ch + 1) * CHUNK, :],
                            idx_sb[:, g_tile * 8 : g_tile * 8 + n_idx // 16],
                            n_idx,
                            n_idx,
                            F,
                            single_packet=False,
                        )
                        tt = 0
                        for wseg, q, nt in segs[ch]:
                            for k in range(nt):
                                oh = oh_p.tile([128, 128], oh_dt)
                                gt = g_tile + tt + k
                                # oh[e, j] = (iota_j == slot_e) * u[dst_e]
                                nc.vector.tensor_scalar(
                                    oh[:],
                                    iota_sb[:],
                                    slots_sb[:, gt : gt + 1],
                                    uvals_sb[:, gt : gt + 1],
                                    mybir.AluOpType.is_equal,
                                    mybir.AluOpType.mult,
                                )
                                nc.tensor.matmul(
                                    psums[wseg][:],
                                    lhsT=msgs[:, (tt + k) * 128 : (tt + k + 1) * 128],
                                    rhs=oh[:],
                                    start=(first_touch[wseg] == (ch, wseg) and k == 0),
                                    stop=(last_touch[wseg] == (ch, wseg) and k == nt - 1),
                                )
                            tt += nt
                        g_tile += seg_tiles
                    # acc[:, window cols] += psum_w
                    for w, pt in psums.items():
                        nc.vector.tensor_tensor(
                            out=acc[:, w * 128 : w * 128 + 128],
                            in0=acc[:, w * 128 : w * 128 + 128],
                            in1=pt[:],
                            op=mybir.AluOpType.add,
                        )
                assert g_tile == T

                # tail: out^T = W^T @ acc + (x^T + b); u[dst] already folded
                # into the one-hot values and the ysT init
                SL = 512
                for s0 in range(0, S, SL):
                    n = min(SL, S - s0)
                    sl = slice(s0, s0 + n)
                    xs_t = fin_p.tile([128, SL], f32, tag="xs")
                    nc.sync.dma_start(xs_t[:, :n], xsT_d[:, sl])
                    pf = fpsum_p.tile([128, SL], f32)
                    nc.tensor.matmul(pf[:, :n], lhsT=w_sb[:], rhs=acc[:, sl],
                                     start=True, stop=True)
                    ot = fin_p.tile([128, SL], f32, tag="ot")
                    nc.vector.tensor_tensor(
                        out=ot[:, :n], in0=pf[:, :n], in1=xs_t[:, :n],
                        op=mybir.AluOpType.add,
                    )
                    nc.sync.dma_start(out_d[:, sl], ot[:, :n])

    nc.compile()
    return nc


_PROGRAM_CACHE = {}


def _get_program(T, sched_key, sched):
    key = (T, sched_key)
    if key not in _PROGRAM_CACHE:
        _PROGRAM_CACHE[key] = _build_program(T, sched)
    return _PROGRAM_CACHE[key]


def _prepare(x, edge_index, W, b):
    x = np.asarray(x, dtype=np.float32)
    edge_index = np.asarray(edge_index)
    W = np.asarray(W, dtype=np.float32)
    b = np.asarray(b, dtype=np.float32)

    u, n_tiles, sched, T, idx16, slots, uvals, perm = _host_plan(edge_index)

    import ml_dtypes
    np_msgs = np.float32 if MSGS_DT == "float32" else ml_dtypes.bfloat16
    np_oh = np.float32 if OH_DT == "float32" else ml_dtypes.bfloat16
    y = np.zeros((NPAD, F), dtype=np_msgs)
    y[:N_NODES] = (u[:, None] * x).astype(np_msgs)

    iota = np.tile(np.arange(128, dtype=np.float32), (128, 1))

    # staged per-core rows follow the dst permutation; -1 slots stay zero
    u_ext = np.concatenate([u, [0.0]]).astype(np.float32)
    x_ext = np.concatenate([x, np.zeros((1, F), np.float32)], axis=0)
    # acc init carries the self-loop term already scaled by u[dst]: u^2 * x
    ys_ext = u_ext[:, None] ** 2 * x_ext

    in_maps = []
    for c in range(N_CORES):
        rows = perm[c]  # global dst ids at this core's slot positions (-1 empty)
        # idx stream position i -> [i % 16, i // 16]; 16-row block
        # replicated 8x along partitions (one copy per Q7 core group)
        idx_c = np.tile(idx16[c].reshape(-1, 16).T, (8, 1)).copy()  # [128, T*8]
        slots_c = slots[c].reshape(T, 128).T.copy()  # [128, T]
        ysT = ys_ext[rows].T.copy()
        xsT = (x_ext[rows] + b[None, :]).T.copy()
        in_maps.append(
            {
                "y": y,
                "idx16": idx_c,
                "slots": slots_c.astype(np.float32),
                "uvals": uvals[c].reshape(T, 128).T.copy().astype(np.float32),
                "iota": iota,
                "ysT": np.ascontiguousarray(ysT),
                "xsT": np.ascontiguousarray(xsT),
                "W": W,
            }
        )

    sched_key = tuple(
        (g, tuple(tuple(tuple(t) for t in seg) for seg in segs))
        for g, segs, _, _ in sched
    )
    nc = _get_program(T, sched_key, sched)
    global _LAST_PERM
    _LAST_PERM = perm
    return nc, in_maps


_LAST_PERM = None


def _unshard(results, perm=None):
    if perm is None:
        perm = _LAST_PERM
    out = np.empty((N_NODES, F), dtype=np.float32)
    for c in range(N_CORES):
        rows = perm[c]
        valid = rows >= 0
        out[rows[valid]] = results[c]["outT"].T[valid]
    return out


def kernel(x, edge_index, W, b):
    from concourse.bass_utils import run_bass_kernel_spmd

    nc, in_maps = _prepare(x, edge_index, W, b)
    res = run_bass_kernel_spmd(nc, in_maps, list(range(N_CORES)))
    return _unshard(res.results)


if __name__ == "__main__":
    rng = np.random.default_rng(0)
    x = rng.standard_normal((N_NODES, F), dtype=np.float32)
    ei = rng.integers(0, N_NODES, size=(2, 1600000)).astype(np.int64)
    W = rng.standard_normal((F, F), dtype=np.float32) / np.sqrt(F)
    b = np.zeros(F, dtype=np.float32)
    out = kernel(x=x, edge_index=ei, W=W, b=b)
    print(out.shape, out.dtype)



# revision 4
# speedup vs baseline: 1.2633x; 1.2633x over previous
"""GCNConvSC (residual + GCNConv) Trainium2 Bass kernel, 8-core SPMD.

Math (matches the PyG-style reference):
    deg[v]  = indeg_with_selfloop(v)          (count of v in dst, +1)
    u       = deg^{-1/2}
    y       = u[:,None] * x                   (pre-scaled node features, fp8)
    z[v]    = sum_{e: dst_e = v} y[src_e] * u[v]   (via one-hot matmuls)
    out[v]  = x[v] + b + (z[v] + u[v]^2 * x[v]) @ W

Pipeline per core (dst nodes range-partitioned, S=12544 slots, 98 windows
of 128):
  - y stored in HBM as fp8 e4m3 rows padded to a 256B stride; per-edge rows
    are fetched with a raw InstDMAGatherAnt (elem_size=128, elem_step=256),
    i.e. 128B descriptors, which the DMA cost model prices at half the
    256B-descriptor rate.  Edges are bucketed by (window-group, src-chunk,
    window) with int16 chunk-local indices (4 chunks of 25024 rows).
  - Aggregation: per 128-edge tile a bf16 one-hot (iota==slot)*u[dst] is
    built on DVE (4x perf mode) and matmul'd (fp8 lhsT x bf16 rhs) into a
    PSUM bank quarter for the edge's dst window.
  - The self-loop term ys = u^2*x and the residual xs = x + b are seeded
    into PSUM by identity-rhs matmuls (lhsT = node-major bf16 tiles), so
    the SBUF accumulator is write-once and flushes are plain Activation-
    engine PSUM->SBUF copies (DVE stays free for one-hots).
  - Tail: out^T = W^T @ acc accumulated on top of the xs seed, copied to
    bf16 and stored.
"""

import sys

sys.path.insert(0, "/opt/trn_rl_repo")

import numpy as np

N_NODES = 100000
F = 128
N_CORES = 8
S = 12544            # dst slots per core (98 windows of 128)
WN = 98              # windows per core
WG = 16              # windows per PSUM group (4 banks of 4 windows)
N_CHUNKS = 4
CHUNK = 25024        # gather-source rows per chunk (int16-safe)
NPAD = N_CHUNKS * CHUNK  # 100096 padded node rows for y
YSTRIDE = 256        # fp8 row stride in bytes (DMA desc stride granularity)


def _host_plan(edge_index):
    """Sort/bucket edges per core; emit the shared SPMD schedule plus
    per-core gather-index and slot arrays."""
    src = np.asarray(edge_index[0], dtype=np.int64)
    dst = np.asarray(edge_index[1], dtype=np.int64)

    deg_e = np.bincount(dst, minlength=N_NODES)
    u = (1.0 / np.sqrt(deg_e.astype(np.float64) + 1.0)).astype(np.float32)

    # Deal dsts snake-wise by descending degree across cores: every core's
    # position-p dst has ~the same degree, so per-(chunk, window) counts are
    # nearly equal across cores and the shared max-based schedule pads little.
    order = np.argsort(-deg_e, kind="stable")
    i = np.arange(N_NODES)
    blk, lane = i // N_CORES, i % N_CORES
    core_i = np.where(blk % 2 == 0, lane, N_CORES - 1 - lane)
    perm = np.full((N_CORES, S), -1, dtype=np.int64)
    perm[core_i, blk] = order
    core_of_node = np.empty(N_NODES, dtype=np.int64)
    pos_of_node = np.empty(N_NODES, dtype=np.int64)
    core_of_node[order] = core_i
    pos_of_node[order] = blk

    core_of = core_of_node[dst]
    pos_e_all = pos_of_node[dst]
    u_e_all = u[dst]
    chunk_of = src // CHUNK

    per_core = []
    counts = np.zeros((N_CORES, N_CHUNKS, WN), dtype=np.int64)
    for c in range(N_CORES):
        m = core_of == c
        es, pos_e, ue = src[m], pos_e_all[m], u_e_all[m]
        ch = chunk_of[m]
        w = pos_e // 128
        slot = pos_e % 128
        wg = w // WG
        so = np.lexsort((w, ch, wg))
        es, slot, ch, w, ue = es[so], slot[so], ch[so], w[so], ue[so]
        np.add.at(counts[c], (ch, w), 1)
        per_core.append((es, slot, ch, w, ue))

    # shared schedule: tiles per (chunk, window) = max over cores; windows
    # with zero edges need no tiles (their PSUM quarter is seeded anyway)
    n_tiles = (counts.max(axis=0) + 127) // 128  # [N_CHUNKS, WN]

    n_wg = (WN + WG - 1) // WG
    sched = []  # (g, segs) with segs[ch] = [(window, ntiles), ...]
    T = 0
    for g in range(n_wg):
        ws = range(g * WG, min((g + 1) * WG, WN))
        segs = []
        for ch in range(N_CHUNKS):
            tl = [(w, int(n_tiles[ch, w])) for w in ws if n_tiles[ch, w] > 0]
            segs.append(tl)
        sched.append((g, segs))
        T += int(n_tiles[:, list(ws)].sum())

    # per-core padded edge streams in schedule order
    idx16 = np.zeros((N_CORES, T * 128), dtype=np.int16)
    slots = np.full((N_CORES, T * 128), -1.0, dtype=np.float32)
    uvals = np.zeros((N_CORES, T * 128), dtype=np.float32)
    for c in range(N_CORES):
        es, eslot, ch, w, ue = per_core[c]
        keys = list(zip(w // WG, ch, w))
        run_start = {}
        for i2, k in enumerate(keys):
            if k not in run_start:
                run_start[k] = i2
        run_len = counts[c]
        out_pos = 0
        for g, segs in sched:
            for chp in range(N_CHUNKS):
                for wseg, nt in segs[chp]:
                    cnt = int(run_len[chp, wseg])
                    if cnt > 0:
                        i0 = run_start[(g, chp, wseg)]
                        sl = slice(i0, i0 + cnt)
                        local = (es[sl] - chp * CHUNK).astype(np.int16)
                        idx16[c, out_pos : out_pos + cnt] = local
                        slots[c, out_pos : out_pos + cnt] = eslot[sl].astype(
                            np.float32
                        )
                        uvals[c, out_pos : out_pos + cnt] = ue[sl].astype(np.float32)
                    out_pos += nt * 128
        assert out_pos == T * 128

    return u, n_tiles, sched, T, idx16, slots, uvals, perm


def _raw_gather(gp, mybir, out_ap, in_ap, idxs_ap, num_idxs, elem_size, elem_step):
    """dma_gather (non-transpose, HBM source) without the 256B-multiple
    elem restriction: elem_size may be any size as long as the source row
    STRIDE (elem_step) is a 256B multiple. Mirrors bass.BassGpSimd.dma_gather."""
    import concourse.ap_utils as ap_utils

    assert idxs_ap.dtype == mybir.dt.int16
    assert in_ap.dtype == out_ap.dtype
    stride_bytes = elem_step * mybir.dt.size(in_ap.dtype)
    assert stride_bytes % 256 == 0 and stride_bytes // 256 < 256
    assert ap_utils.ap_is_contiguous(in_ap.ap[1:])
    assert ap_utils.ap_is_contiguous(out_ap.ap[1:])
    assert ap_utils.ap_is_contiguous(idxs_ap.ap[1:])
    assert in_ap.ap[-1][1] == out_ap.ap[-1][1] == elem_size
    assert in_ap.ap[0][0] == elem_step
    _in_ap = gp.lower_ap_dma(in_ap, for_custom_bir_dma=True)
    _idxs_ap = gp.lower_ap(idxs_ap)
    _out_ap = gp.lower_ap(out_ap)
    return gp.add_instruction(
        mybir.InstDMAGatherAnt(
            name=gp.bass.get_next_instruction_name(),
            ins=[*_in_ap, _idxs_ap, gp.lower_val_access(gp.to_reg(num_idxs))],
            outs=[_out_ap],
            transpose=False,
            num_idxs=num_idxs,
            elem_size=elem_size,
            stride_bytes_256=stride_bytes // 256,
            gen_mode=0,
            single_packet=False,
            queue_num=0,
            sbuf_tokens_per_rank=0,
            sbuf_free_dim_per_rank=0,
            sbuf_free_dim_pad_per_rank=0,
            sbuf_byte_offset=0,
        )
    )


def _build_program(T, sched):
    import concourse.bacc as bacc
    import concourse.mybir as mybir
    from concourse import tile

    f32 = mybir.dt.float32
    bf16 = mybir.dt.bfloat16
    fp8 = mybir.dt.float8e4

    nc = bacc.Bacc(
        "TRN2",
        target_bir_lowering=False,
        debug=False,
        enable_asserts=True,
        num_devices=N_CORES,
    )

    y_d = nc.dram_tensor("y8", [NPAD, YSTRIDE], fp8, kind="ExternalInput").ap()
    idx_d = nc.dram_tensor("idx16", [128, T * 8], mybir.dt.int16, kind="ExternalInput").ap()
    slots_d = nc.dram_tensor("slots", [128, T], f32, kind="ExternalInput").ap()
    uvals_d = nc.dram_tensor("uvals", [128, T], f32, kind="ExternalInput").ap()
    iota_d = nc.dram_tensor("iota", [128, 128], bf16, kind="ExternalInput").ap()
    ident_d = nc.dram_tensor("ident", [128, 128], bf16, kind="ExternalInput").ap()
    ysnm_d = nc.dram_tensor("ysnm", [128, WN * F], bf16, kind="ExternalInput").ap()
    xsnm_d = nc.dram_tensor("xsnm", [128, WN * F], bf16, kind="ExternalInput").ap()
    w_d = nc.dram_tensor("W", [F, F], f32, kind="ExternalInput").ap()
    out_d = nc.dram_tensor("outT", [128, S], bf16, kind="ExternalOutput").ap()

    # last chunk with tiles, per window (for matmul stop flags); -1 = none
    last_ch = {}
    for g, segs in sched:
        for ch in range(N_CHUNKS):
            for w, nt in segs[ch]:
                last_ch[w] = ch

    with tile.TileContext(nc) as tc:
        with (
            tc.tile_pool(name="const", bufs=1) as const_p,
            tc.tile_pool(name="acc", bufs=1) as acc_p,
            tc.tile_pool(name="msgs", bufs=4) as msgs_p,
            tc.tile_pool(name="oh", bufs=8) as oh_p,
            tc.tile_pool(name="psum", bufs=8, space="PSUM") as psum_p,
            tc.tile_pool(name="fin", bufs=3) as fin_p,
        ):
            idx_sb = const_p.tile([128, T * 8], mybir.dt.int16)
            slots_sb = const_p.tile([128, T], f32)
            uvals_sb = const_p.tile([128, T], f32)
            iota_sb = const_p.tile([128, 128], bf16)
            ident_sb = const_p.tile([128, 128], bf16)
            ysnm_sb = const_p.tile([128, WN * F], bf16)
            xsnm_sb = const_p.tile([128, WN * F], bf16)
            w_sb = const_p.tile([F, F], f32)
            acc = acc_p.tile([128, S], f32)

            nc.sync.dma_start(idx_sb[:], idx_d[:])
            nc.sync.dma_start(slots_sb[:], slots_d[:])
            nc.sync.dma_start(uvals_sb[:], uvals_d[:])
            nc.sync.dma_start(iota_sb[:], iota_d[:])
            nc.sync.dma_start(ident_sb[:], ident_d[:])
            nc.sync.dma_start(ysnm_sb[:], ysnm_d[:])
            nc.sync.dma_start(xsnm_sb[:], xsnm_d[:])
            nc.sync.dma_start(w_sb[:], w_d[:])

            g_tile = 0  # global tile cursor
            for g, segs in sched:
                ws = list(range(g * WG, min((g + 1) * WG, WN)))
                nbank = (len(ws) + 3) // 4
                banks = [psum_p.tile([128, 512], f32, tag="psum", name=f"ps_g{g}_{b}")
                         for b in range(nbank)]

                def wslice(w):
                    wl = w - g * WG
                    return banks[wl // 4][:, (wl % 4) * 128 : (wl % 4) * 128 + 128]

                # seed each window's PSUM quarter with ys (self-loop term).
                # start=True zeroes the whole 2KB bank, so only the FIRST
                # matmul touching a bank may set it; later seeds add onto
                # the zeroed bank with start=False.
                for w in ws:
                    wl = w - g * WG
                    nc.tensor.matmul(
                        wslice(w),
                        lhsT=ysnm_sb[:, w * F : (w + 1) * F],
                        rhs=ident_sb[:],
                        start=(wl % 4 == 0),
                        stop=w not in last_ch,
                    )
                for ch in range(N_CHUNKS):
                    seg_tiles = sum(nt for (_, nt) in segs[ch])
                    if seg_tiles == 0:
                        continue
                    n_idx = seg_tiles * 128
                    msgs = msgs_p.tile([128, seg_tiles * 128], fp8, tag="msgs")
                    m3 = msgs[:].rearrange("p (b f) -> p b f", f=F)
                    _raw_gather(
                        nc.gpsimd, mybir, m3,
                        y_d[ch * CHUNK : (ch + 1) * CHUNK, 0:F],
                        idx_sb[:, g_tile * 8 : g_tile * 8 + n_idx // 16],
                        n_idx, F, YSTRIDE,
                    )
                    tt = 0
                    for wseg, nt in segs[ch]:
                        for k in range(nt):
                            oh = oh_p.tile([128, 128], bf16)
                            gt = g_tile + tt + k
                            # oh[e, j] = (iota_j == slot_e) * u[dst_e]
                            nc.vector.tensor_scalar(
                                oh[:],
                                iota_sb[:],
                                slots_sb[:, gt : gt + 1],
                                uvals_sb[:, gt : gt + 1],
                                mybir.AluOpType.is_equal,
                                mybir.AluOpType.mult,
                            )
                            nc.tensor.matmul(
                                wslice(wseg),
                                lhsT=msgs[:, (tt + k) * 128 : (tt + k + 1) * 128],
                                rhs=oh[:],
                                start=False,
                                stop=(last_ch[wseg] == ch and k == nt - 1),
                            )
                        tt += nt
                    g_tile += seg_tiles
                # flush: Act copies PSUM banks into the (write-once) acc
                for b in range(nbank):
                    c0 = (g * WG + b * 4) * 128
                    ncols = min(512, S - c0)
                    nc.scalar.copy(out=acc[:, c0 : c0 + ncols], in_=banks[b][:, :ncols])
            assert g_tile == T

            # tail: out^T = W^T @ acc + xs (xs = x + b seeded via identity)
            SL = 512
            for s0 in range(0, S, SL):
                n = min(SL, S - s0)
                pf = psum_p.tile([128, SL], f32, tag="psum", name=f"pf_{s0}")
                for j in range((n + 127) // 128):
                    w = s0 // 128 + j
                    nc.tensor.matmul(
                        pf[:, j * 128 : j * 128 + 128],
                        lhsT=xsnm_sb[:, w * F : (w + 1) * F],
                        rhs=ident_sb[:],
                        start=(j == 0),
                        stop=False,
                    )
                nc.tensor.matmul(pf[:, :n], lhsT=w_sb[:], rhs=acc[:, s0 : s0 + n],
                                 start=False, stop=True)
                ot = fin_p.tile([128, SL], bf16, tag="ot")
                nc.scalar.copy(out=ot[:, :n], in_=pf[:, :n])
                nc.sync.dma_start(out_d[:, s0 : s0 + n], ot[:, :n])

    nc.compile()
    return nc


_PROGRAM_CACHE = {}


def _get_program(T, sched):
    key = (T, tuple((g, tuple(tuple(seg) for seg in segs)) for g, segs in sched))
    if key not in _PROGRAM_CACHE:
        _PROGRAM_CACHE[key] = _build_program(T, sched)
    return _PROGRAM_CACHE[key]


def _prepare(x, edge_index, W, b):
    x = np.asarray(x, dtype=np.float32)
    edge_index = np.asarray(edge_index)
    W = np.asarray(W, dtype=np.float32)
    b = np.asarray(b, dtype=np.float32)

    u, n_tiles, sched, T, idx16, slots, uvals, perm = _host_plan(edge_index)

    import ml_dtypes
    bf = ml_dtypes.bfloat16
    f8 = ml_dtypes.float8_e4m3
    y8 = np.zeros((NPAD, YSTRIDE), dtype=f8)
    y8[:N_NODES, :F] = (u[:, None] * x).astype(f8)

    iota = np.tile(np.arange(128, dtype=np.float32), (128, 1)).astype(bf)
    ident = np.eye(128, dtype=np.float32).astype(bf)

    u_ext = np.concatenate([u, [0.0]]).astype(np.float32)
    x_ext = np.concatenate([x, np.zeros((1, F), np.float32)], axis=0)
    ys_ext = u_ext[:, None] ** 2 * x_ext
    xs_ext = x_ext + b[None, :]

    in_maps = []
    for c in range(N_CORES):
        rows = perm[c]
        idx_c = np.tile(idx16[c].reshape(-1, 16).T, (8, 1)).copy()  # [128, T*8]
        slots_c = slots[c].reshape(T, 128).T.copy()
        uvals_c = uvals[c].reshape(T, 128).T.copy()
        # node-major per-window tiles: [slot-partition, window, feature]
        ysnm = ys_ext[rows].astype(bf).reshape(WN, 128, F).transpose(1, 0, 2)
        xsnm = xs_ext[rows].astype(bf).reshape(WN, 128, F).transpose(1, 0, 2)
        in_maps.append(
            {
                "y8": y8,
                "idx16": idx_c,
                "slots": slots_c.astype(np.float32),
                "uvals": uvals_c.astype(np.float32),
                "iota": iota,
                "ident": ident,
                "ysnm": np.ascontiguousarray(ysnm.reshape(128, WN * F)),
                "xsnm": np.ascontiguousarray(xsnm.reshape(128, WN * F)),
                "W": W,
            }
        )

    nc = _get_program(T, sched)
    global _LAST_PERM
    _LAST_PERM = perm
    return nc, in_maps


_LAST_PERM = None


def _unshard(results, perm=None):
    if perm is None:
        perm = _LAST_PERM
    out = np.empty((N_NODES, F), dtype=np.float32)
    for c in range(N_CORES):
        rows = perm[c]
        valid = rows >= 0
        out[rows[valid]] = results[c]["outT"].T.astype(np.float32)[valid]
    return out


def kernel(x, edge_index, W, b):
    from concourse.bass_utils import run_bass_kernel_spmd

    nc, in_maps = _prepare(x, edge_index, W, b)
    res = run_bass_kernel_spmd(nc, in_maps, list(range(N_CORES)))
    return _unshard(res.results)


if __name__ == "__main__":
    rng = np.random.default_rng(0)
    x = rng.standard_normal((N_NODES, F), dtype=np.float32)
    ei = rng.integers(0, N_NODES, size=(2, 1600000)).astype(np.int64)
    W = rng.standard_normal((F, F), dtype=np.float32) / np.sqrt(F)
    b = np.zeros(F, dtype=np.float32)
    out = kernel(x=x, edge_index=ei, W=W, b=b)
    print(out.shape, out.dtype)


# revision 12
# speedup vs baseline: 1.6941x; 1.3410x over previous
"""GCNConvSC (residual + GCNConv) Trainium2 Bass kernel, 8-core SPMD.

Math (matches the PyG-style reference):
    deg[v]  = indeg_with_selfloop(v)          (count of v in dst, +1)
    u       = deg^{-1/2}
    y       = u[:,None] * x                   (pre-scaled node features, fp8)
    z[v]    = sum_{e: dst_e = v} y[src_e] * u[v]   (via one-hot matmuls)
    out[v]  = x[v] + b + (z[v] + u[v]^2 * x[v]) @ W

Pipeline per core (dst nodes range-partitioned, S=12544 slots, 98 windows
of 128):
  - y stored in HBM as fp8 e4m3 rows padded to a 256B stride; per-edge rows
    are fetched with a raw InstDMAGatherAnt (elem_size=128, elem_step=256),
    i.e. 128B descriptors, which the DMA cost model prices at half the
    256B-descriptor rate.  Edges are bucketed by (window-group, src-chunk,
    window) with int16 chunk-local indices (4 chunks of 25024 rows).
  - Aggregation: per 128-edge tile a bf16 one-hot (iota==slot)*u[dst] is
    built on DVE (4x perf mode) and matmul'd (fp8 lhsT x bf16 rhs) into a
    PSUM bank quarter for the edge's dst window.
  - The self-loop term ys = u^2*x and the residual xs = x + b are seeded
    into PSUM by identity-rhs matmuls (lhsT = node-major bf16 tiles), so
    the SBUF accumulator is write-once and flushes are plain Activation-
    engine PSUM->SBUF copies (DVE stays free for one-hots).
  - Tail: out^T = W^T @ acc accumulated on top of the xs seed, copied to
    bf16 and stored.
"""

import sys

sys.path.insert(0, "/opt/trn_rl_repo")

import numpy as np

N_NODES = 100000
F = 128
N_CORES = 8
S = 12544            # dst slots per core (98 windows of 128)
WN = 98              # windows per core
WG = 16              # windows per PSUM group (4 banks of 4 windows)
N_CHUNKS = 4
CHUNK = 25024        # gather-source rows per chunk (int16-safe)
NPAD = N_CHUNKS * CHUNK  # 100096 padded node rows for y
YSTRIDE = 256        # fp8 row stride in bytes (DMA desc stride granularity)


def _host_plan(edge_index):
    """Sort/bucket edges per core; emit the shared SPMD schedule plus
    per-core gather-index and slot arrays."""
    src = np.asarray(edge_index[0], dtype=np.int64)
    dst = np.asarray(edge_index[1], dtype=np.int64)

    deg_e = np.bincount(dst, minlength=N_NODES)
    u = (1.0 / np.sqrt(deg_e.astype(np.float64) + 1.0)).astype(np.float32)

    # Deal dsts snake-wise by descending degree across cores: every core's
    # position-p dst has ~the same degree, so per-(chunk, window) counts are
    # nearly equal across cores and the shared max-based schedule pads little.
    order = np.argsort(-deg_e, kind="stable")
    i = np.arange(N_NODES)
    blk, lane = i // N_CORES, i % N_CORES
    core_i = np.where(blk % 2 == 0, lane, N_CORES - 1 - lane)
    perm = np.full((N_CORES, S), -1, dtype=np.int64)
    perm[core_i, blk] = order
    core_of_node = np.empty(N_NODES, dtype=np.int64)
    pos_of_node = np.empty(N_NODES, dtype=np.int64)
    core_of_node[order] = core_i
    pos_of_node[order] = blk

    core_of = core_of_node[dst]
    pos_e_all = pos_of_node[dst]
    u_e_all = u[dst]
    chunk_of = src // CHUNK

    per_core = []
    counts = np.zeros((N_CORES, N_CHUNKS, WN), dtype=np.int64)
    for c in range(N_CORES):
        m = core_of == c
        es, pos_e, ue = src[m], pos_e_all[m], u_e_all[m]
        ch = chunk_of[m]
        w = pos_e // 128
        slot = pos_e % 128
        wg = w // WG
        so = np.lexsort((w, ch, wg))
        es, slot, ch, w, ue = es[so], slot[so], ch[so], w[so], ue[so]
        np.add.at(counts[c], (ch, w), 1)
        per_core.append((es, slot, ch, w, ue))

    # shared schedule: tiles per (chunk, window) = max over cores; windows
    # with zero edges need no tiles (their PSUM quarter is seeded anyway)
    n_tiles = (counts.max(axis=0) + 127) // 128  # [N_CHUNKS, WN]

    n_wg = (WN + WG - 1) // WG
    sched = []  # (g, segs) with segs[ch] = [(window, ntiles), ...]
    T = 0
    for g in range(n_wg):
        ws = range(g * WG, min((g + 1) * WG, WN))
        segs = []
        for ch in range(N_CHUNKS):
            tl = [(w, int(n_tiles[ch, w])) for w in ws if n_tiles[ch, w] > 0]
            segs.append(tl)
        sched.append((g, segs))
        T += int(n_tiles[:, list(ws)].sum())

    # per-core padded edge streams in schedule order
    idx16 = np.zeros((N_CORES, T * 128), dtype=np.int16)
    slots = np.full((N_CORES, T * 128), -1.0, dtype=np.float32)
    uvals = np.zeros((N_CORES, T * 128), dtype=np.float32)
    for c in range(N_CORES):
        es, eslot, ch, w, ue = per_core[c]
        keys = list(zip(w // WG, ch, w))
        run_start = {}
        for i2, k in enumerate(keys):
            if k not in run_start:
                run_start[k] = i2
        run_len = counts[c]
        out_pos = 0
        for g, segs in sched:
            for chp in range(N_CHUNKS):
                for wseg, nt in segs[chp]:
                    cnt = int(run_len[chp, wseg])
                    if cnt > 0:
                        i0 = run_start[(g, chp, wseg)]
                        sl = slice(i0, i0 + cnt)
                        local = (es[sl] - chp * CHUNK).astype(np.int16)
                        idx16[c, out_pos : out_pos + cnt] = local
                        slots[c, out_pos : out_pos + cnt] = eslot[sl].astype(
                            np.float32
                        )
                        uvals[c, out_pos : out_pos + cnt] = ue[sl].astype(np.float32)
                    out_pos += nt * 128
        assert out_pos == T * 128

    return u, n_tiles, sched, T, idx16, slots, uvals, perm


def _raw_gather(gp, mybir, out_ap, in_ap, idxs_ap, num_idxs, elem_size, elem_step):
    """dma_gather (non-transpose, HBM source) without the 256B-multiple
    elem restriction: elem_size may be any size as long as the source row
    STRIDE (elem_step) is a 256B multiple. Mirrors bass.BassGpSimd.dma_gather."""
    import concourse.ap_utils as ap_utils

    assert idxs_ap.dtype == mybir.dt.int16
    assert in_ap.dtype == out_ap.dtype
    stride_bytes = elem_step * mybir.dt.size(in_ap.dtype)
    assert stride_bytes % 256 == 0 and stride_bytes // 256 < 256
    assert ap_utils.ap_is_contiguous(in_ap.ap[1:])
    assert ap_utils.ap_is_contiguous(out_ap.ap[1:])
    assert ap_utils.ap_is_contiguous(idxs_ap.ap[1:])
    assert in_ap.ap[-1][1] == out_ap.ap[-1][1] == elem_size
    assert in_ap.ap[0][0] == elem_step
    _in_ap = gp.lower_ap_dma(in_ap, for_custom_bir_dma=True)
    _idxs_ap = gp.lower_ap(idxs_ap)
    _out_ap = gp.lower_ap(out_ap)
    return gp.add_instruction(
        mybir.InstDMAGatherAnt(
            name=gp.bass.get_next_instruction_name(),
            ins=[*_in_ap, _idxs_ap, gp.lower_val_access(gp.to_reg(num_idxs))],
            outs=[_out_ap],
            transpose=False,
            num_idxs=num_idxs,
            elem_size=elem_size,
            stride_bytes_256=stride_bytes // 256,
            gen_mode=0,
            single_packet=False,
            queue_num=0,
            sbuf_tokens_per_rank=0,
            sbuf_free_dim_per_rank=0,
            sbuf_free_dim_pad_per_rank=0,
            sbuf_byte_offset=0,
        )
    )


def _build_program(T, sched):
    import concourse.bacc as bacc
    import concourse.mybir as mybir
    from concourse import tile

    f32 = mybir.dt.float32
    bf16 = mybir.dt.bfloat16
    fp8 = mybir.dt.float8e4

    nc = bacc.Bacc(
        "TRN2",
        target_bir_lowering=False,
        debug=False,
        enable_asserts=True,
        num_devices=N_CORES,
    )

    y_d = nc.dram_tensor("y8", [NPAD, YSTRIDE], fp8, kind="ExternalInput").ap()
    idx_d = nc.dram_tensor("idx16", [128, T * 8], mybir.dt.int16, kind="ExternalInput").ap()
    slots_d = nc.dram_tensor("slots", [128, T], f32, kind="ExternalInput").ap()
    uvals_d = nc.dram_tensor("uvals", [128, T], f32, kind="ExternalInput").ap()
    iota_d = nc.dram_tensor("iota", [128, 128], bf16, kind="ExternalInput").ap()
    ident_d = nc.dram_tensor("ident", [128, 128], bf16, kind="ExternalInput").ap()
    ysnm_d = nc.dram_tensor("ysnm", [128, WN * F], bf16, kind="ExternalInput").ap()
    xsnm_d = nc.dram_tensor("xsnm", [128, WN * F], bf16, kind="ExternalInput").ap()
    w_d = nc.dram_tensor("W", [F, F], f32, kind="ExternalInput").ap()
    out_d = nc.dram_tensor("outT", [128, S], bf16, kind="ExternalOutput").ap()

    # last chunk with tiles, per window (for matmul stop flags); -1 = none
    last_ch = {}
    for g, segs in sched:
        for ch in range(N_CHUNKS):
            for w, nt in segs[ch]:
                last_ch[w] = ch

    with tile.TileContext(nc) as tc:
        with (
            tc.tile_pool(name="const", bufs=1) as const_p,
            tc.tile_pool(name="acc", bufs=1) as acc_p,
            tc.tile_pool(name="msgs", bufs=4) as msgs_p,
            tc.tile_pool(name="oh", bufs=4) as oh_p,
            tc.tile_pool(name="psum", bufs=8, space="PSUM") as psum_p,
            tc.tile_pool(name="fin", bufs=3) as fin_p,
        ):
            idx_sb = const_p.tile([128, T * 8], mybir.dt.int16)
            slots_sb = const_p.tile([128, T], f32)
            uvals_sb = const_p.tile([128, T], f32)
            iota_sb = const_p.tile([128, 128], bf16)
            ident_sb = const_p.tile([128, 128], bf16)
            ysnm_sb = const_p.tile([128, WN * F], bf16)
            xsnm_sb = const_p.tile([128, WN * F], bf16)
            w_sb = const_p.tile([F, F], f32)
            acc = acc_p.tile([128, S], f32)

            nc.sync.dma_start(idx_sb[:], idx_d[:])
            nc.sync.dma_start(slots_sb[:], slots_d[:])
            nc.sync.dma_start(uvals_sb[:], uvals_d[:])
            nc.sync.dma_start(iota_sb[:], iota_d[:])
            nc.sync.dma_start(ident_sb[:], ident_d[:])
            nc.sync.dma_start(ysnm_sb[:], ysnm_d[:])
            nc.sync.dma_start(xsnm_sb[:], xsnm_d[:])
            nc.sync.dma_start(w_sb[:], w_d[:])

            g_tile = 0  # global tile cursor
            for g, segs in sched:
                ws = list(range(g * WG, min((g + 1) * WG, WN)))
                nbank = (len(ws) + 3) // 4
                banks = [psum_p.tile([128, 512], f32, tag="psum", name=f"ps_g{g}_{b}")
                         for b in range(nbank)]

                def wslice(w):
                    wl = w - g * WG
                    return banks[wl // 4][:, (wl % 4) * 128 : (wl % 4) * 128 + 128]

                # seed each window's PSUM quarter with ys (self-loop term).
                # start=True zeroes the whole 2KB bank, so only the FIRST
                # matmul touching a bank may set it; later seeds add onto
                # the zeroed bank with start=False.
                for w in ws:
                    wl = w - g * WG
                    nc.tensor.matmul(
                        wslice(w),
                        lhsT=ysnm_sb[:, w * F : (w + 1) * F],
                        rhs=ident_sb[:],
                        start=(wl % 4 == 0),
                        stop=w not in last_ch,
                    )
                for ch in range(N_CHUNKS):
                    seg_tiles = sum(nt for (_, nt) in segs[ch])
                    if seg_tiles == 0:
                        continue
                    n_idx = seg_tiles * 128
                    msgs = msgs_p.tile([128, seg_tiles * 128], fp8, tag="msgs")
                    m3 = msgs[:].rearrange("p (b f) -> p b f", f=F)
                    _raw_gather(
                        nc.gpsimd, mybir, m3,
                        y_d[ch * CHUNK : (ch + 1) * CHUNK, 0:F],
                        idx_sb[:, g_tile * 8 : g_tile * 8 + n_idx // 16],
                        n_idx, F, YSTRIDE,
                    )
                    # one-hots built into 8-tile super-tiles so the tile
                    # framework batches buffer-reuse waits (PE consumes
                    # in-order, so one wait covers the whole super-tile)
                    OHB = 8
                    flat = [(wseg, k, nt) for wseg, nt in segs[ch]
                            for k in range(nt)]
                    oh_sup = None
                    for ti, (wseg, k, nt) in enumerate(flat):
                        ob = ti % OHB
                        if ob == 0:
                            nb = min(OHB, len(flat) - ti)
                            oh_sup = oh_p.tile([128, OHB * 128], bf16)
                        gt = g_tile + ti
                        # oh[e, j] = (iota_j == slot_e) * u[dst_e]
                        nc.vector.tensor_scalar(
                            oh_sup[:, ob * 128 : ob * 128 + 128],
                            iota_sb[:],
                            slots_sb[:, gt : gt + 1],
                            uvals_sb[:, gt : gt + 1],
                            mybir.AluOpType.is_equal,
                            mybir.AluOpType.mult,
                        )
                        nc.tensor.matmul(
                            wslice(wseg),
                            lhsT=msgs[:, ti * 128 : (ti + 1) * 128],
                            rhs=oh_sup[:, ob * 128 : ob * 128 + 128],
                            start=False,
                            stop=(last_ch[wseg] == ch and k == nt - 1),
                        )
                    g_tile += seg_tiles
                # flush: Act copies PSUM banks into the (write-once) acc
                for b in range(nbank):
                    c0 = (g * WG + b * 4) * 128
                    ncols = min(512, S - c0)
                    nc.scalar.copy(out=acc[:, c0 : c0 + ncols], in_=banks[b][:, :ncols])
            assert g_tile == T

            # tail: out^T = W^T @ acc + xs (xs = x + b seeded via identity)
            SL = 512
            for s0 in range(0, S, SL):
                n = min(SL, S - s0)
                pf = psum_p.tile([128, SL], f32, tag="psum", name=f"pf_{s0}")
                for j in range((n + 127) // 128):
                    w = s0 // 128 + j
                    nc.tensor.matmul(
                        pf[:, j * 128 : j * 128 + 128],
                        lhsT=xsnm_sb[:, w * F : (w + 1) * F],
                        rhs=ident_sb[:],
                        start=(j == 0),
                        stop=False,
                    )
                nc.tensor.matmul(pf[:, :n], lhsT=w_sb[:], rhs=acc[:, s0 : s0 + n],
                                 start=False, stop=True)
                ot = fin_p.tile([128, SL], bf16, tag="ot")
                nc.scalar.copy(out=ot[:, :n], in_=pf[:, :n])
                nc.sync.dma_start(out_d[:, s0 : s0 + n], ot[:, :n])

    nc.compile()
    return nc


_PROGRAM_CACHE = {}


def _get_program(T, sched):
    key = (T, tuple((g, tuple(tuple(seg) for seg in segs)) for g, segs in sched))
    if key not in _PROGRAM_CACHE:
        _PROGRAM_CACHE[key] = _build_program(T, sched)
    return _PROGRAM_CACHE[key]


def _prepare(x, edge_index, W, b):
    x = np.asarray(x, dtype=np.float32)
    edge_index = np.asarray(edge_index)
    W = np.asarray(W, dtype=np.float32)
    b = np.asarray(b, dtype=np.float32)

    u, n_tiles, sched, T, idx16, slots, uvals, perm = _host_plan(edge_index)

    import ml_dtypes
    bf = ml_dtypes.bfloat16
    f8 = ml_dtypes.float8_e4m3
    y8 = np.zeros((NPAD, YSTRIDE), dtype=f8)
    y8[:N_NODES, :F] = (u[:, None] * x).astype(f8)

    iota = np.tile(np.arange(128, dtype=np.float32), (128, 1)).astype(bf)
    ident = np.eye(128, dtype=np.float32).astype(bf)

    u_ext = np.concatenate([u, [0.0]]).astype(np.float32)
    x_ext = np.concatenate([x, np.zeros((1, F), np.float32)], axis=0)
    ys_ext = u_ext[:, None] ** 2 * x_ext
    xs_ext = x_ext + b[None, :]

    in_maps = []
    for c in range(N_CORES):
        rows = perm[c]
        idx_c = np.tile(idx16[c].reshape(-1, 16).T, (8, 1)).copy()  # [128, T*8]
        slots_c = slots[c].reshape(T, 128).T.copy()
        uvals_c = uvals[c].reshape(T, 128).T.copy()
        # node-major per-window tiles: [slot-partition, window, feature]
        ysnm = ys_ext[rows].astype(bf).reshape(WN, 128, F).transpose(1, 0, 2)
        xsnm = xs_ext[rows].astype(bf).reshape(WN, 128, F).transpose(1, 0, 2)
        in_maps.append(
            {
                "y8": y8,
                "idx16": idx_c,
                "slots": slots_c.astype(np.float32),
                "uvals": uvals_c.astype(np.float32),
                "iota": iota,
                "ident": ident,
                "ysnm": np.ascontiguousarray(ysnm.reshape(128, WN * F)),
                "xsnm": np.ascontiguousarray(xsnm.reshape(128, WN * F)),
                "W": W,
            }
        )

    nc = _get_program(T, sched)
    global _LAST_PERM
    _LAST_PERM = perm
    return nc, in_maps


_LAST_PERM = None


def _unshard(results, perm=None):
    if perm is None:
        perm = _LAST_PERM
    out = np.empty((N_NODES, F), dtype=np.float32)
    for c in range(N_CORES):
        rows = perm[c]
        valid = rows >= 0
        out[rows[valid]] = results[c]["outT"].T.astype(np.float32)[valid]
    return out


def kernel(x, edge_index, W, b):
    from concourse.bass_utils import run_bass_kernel_spmd

    nc, in_maps = _prepare(x, edge_index, W, b)
    res = run_bass_kernel_spmd(nc, in_maps, list(range(N_CORES)))
    return _unshard(res.results)


if __name__ == "__main__":
    rng = np.random.default_rng(0)
    x = rng.standard_normal((N_NODES, F), dtype=np.float32)
    ei = rng.integers(0, N_NODES, size=(2, 1600000)).astype(np.int64)
    W = rng.standard_normal((F, F), dtype=np.float32) / np.sqrt(F)
    b = np.zeros(F, dtype=np.float32)
    out = kernel(x=x, edge_index=ei, W=W, b=b)
    print(out.shape, out.dtype)


# revision 19
# speedup vs baseline: 1.8043x; 1.0650x over previous
"""GCNConvSC (residual + GCNConv) Trainium2 Bass kernel, 8-core SPMD.

Math (matches the PyG-style reference):
    deg[v]  = indeg_with_selfloop(v)          (count of v in dst, +1)
    u       = deg^{-1/2}
    y       = u[:,None] * x                   (pre-scaled node features, fp8)
    z[v]    = sum_{e: dst_e = v} y[src_e] * u[v]   (via one-hot matmuls)
    out[v]  = x[v] + b + (z[v] + u[v]^2 * x[v]) @ W

Pipeline per core (dst nodes range-partitioned, S=12544 slots, 98 windows
of 128):
  - y stored in HBM as fp8 e4m3 rows padded to a 256B stride; per-edge rows
    are fetched with a raw InstDMAGatherAnt (elem_size=128, elem_step=256),
    i.e. 128B descriptors, which the DMA cost model prices at half the
    256B-descriptor rate.  Edges are bucketed by (window-group, src-chunk,
    window) with int16 chunk-local indices (4 chunks of 25024 rows).
  - Aggregation: per 128-edge tile a bf16 one-hot (iota==slot)*u[dst] is
    built on DVE (4x perf mode) and matmul'd (fp8 lhsT x bf16 rhs) into a
    PSUM bank quarter for the edge's dst window.
  - The self-loop term ys = u^2*x and the residual xs = x + b are seeded
    into PSUM by identity-rhs matmuls (lhsT = node-major bf16 tiles), so
    the SBUF accumulator is write-once and flushes are plain Activation-
    engine PSUM->SBUF copies (DVE stays free for one-hots).
  - Tail: out^T = W^T @ acc accumulated on top of the xs seed, copied to
    bf16 and stored.
"""

import sys

sys.path.insert(0, "/opt/trn_rl_repo")

import numpy as np

N_NODES = 100000
F = 128
N_CORES = 8
S = 12544            # dst slots per core (98 windows of 128)
WN = 98              # windows per core
WG = 16              # windows per PSUM group (4 banks of 4 windows)
N_CHUNKS = 4
CHUNK = 25024        # gather-source rows per chunk (int16-safe)
NPAD = N_CHUNKS * CHUNK  # 100096 padded node rows for y
YSTRIDE = 256        # fp8 row stride in bytes (DMA desc stride granularity)


def _host_plan(edge_index):
    """Sort/bucket edges per core; emit the shared SPMD schedule plus
    per-core gather-index and slot arrays."""
    src = np.asarray(edge_index[0], dtype=np.int64)
    dst = np.asarray(edge_index[1], dtype=np.int64)

    deg_e = np.bincount(dst, minlength=N_NODES)
    u = (1.0 / np.sqrt(deg_e.astype(np.float64) + 1.0)).astype(np.float32)

    # Deal dsts snake-wise by descending degree across cores: every core's
    # position-p dst has ~the same degree, so per-(chunk, window) counts are
    # nearly equal across cores and the shared max-based schedule pads little.
    order = np.argsort(-deg_e, kind="stable")
    i = np.arange(N_NODES)
    blk, lane = i // N_CORES, i % N_CORES
    core_i = np.where(blk % 2 == 0, lane, N_CORES - 1 - lane)
    perm = np.full((N_CORES, S), -1, dtype=np.int64)
    perm[core_i, blk] = order
    core_of_node = np.empty(N_NODES, dtype=np.int64)
    pos_of_node = np.empty(N_NODES, dtype=np.int64)
    core_of_node[order] = core_i
    pos_of_node[order] = blk

    core_of = core_of_node[dst]
    pos_e_all = pos_of_node[dst]
    u_e_all = u[dst]
    chunk_of = src // CHUNK

    per_core = []
    counts = np.zeros((N_CORES, N_CHUNKS, WN), dtype=np.int64)
    for c in range(N_CORES):
        m = core_of == c
        es, pos_e, ue = src[m], pos_e_all[m], u_e_all[m]
        ch = chunk_of[m]
        w = pos_e // 128
        slot = pos_e % 128
        wg = w // WG
        so = np.lexsort((w, ch, wg))
        es, slot, ch, w, ue = es[so], slot[so], ch[so], w[so], ue[so]
        np.add.at(counts[c], (ch, w), 1)
        per_core.append((es, slot, ch, w, ue))

    # shared schedule: tiles per (chunk, window) = max over cores; windows
    # with zero edges need no tiles (their PSUM quarter is seeded anyway)
    n_tiles = (counts.max(axis=0) + 127) // 128  # [N_CHUNKS, WN]

    n_wg = (WN + WG - 1) // WG
    sched = []  # (g, segs) with segs[ch] = [(window, ntiles), ...]
    T = 0
    for g in range(n_wg):
        ws = range(g * WG, min((g + 1) * WG, WN))
        segs = []
        for ch in range(N_CHUNKS):
            tl = [(w, int(n_tiles[ch, w])) for w in ws if n_tiles[ch, w] > 0]
            segs.append(tl)
        sched.append((g, segs))
        T += int(n_tiles[:, list(ws)].sum())

    # per-core padded edge streams in schedule order
    idx16 = np.zeros((N_CORES, T * 128), dtype=np.int16)
    slots = np.full((N_CORES, T * 128), -1.0, dtype=np.float32)
    uvals = np.zeros((N_CORES, T * 128), dtype=np.float32)
    for c in range(N_CORES):
        es, eslot, ch, w, ue = per_core[c]
        keys = list(zip(w // WG, ch, w))
        run_start = {}
        for i2, k in enumerate(keys):
            if k not in run_start:
                run_start[k] = i2
        run_len = counts[c]
        out_pos = 0
        for g, segs in sched:
            for chp in range(N_CHUNKS):
                for wseg, nt in segs[chp]:
                    cnt = int(run_len[chp, wseg])
                    if cnt > 0:
                        i0 = run_start[(g, chp, wseg)]
                        sl = slice(i0, i0 + cnt)
                        local = (es[sl] - chp * CHUNK).astype(np.int16)
                        idx16[c, out_pos : out_pos + cnt] = local
                        slots[c, out_pos : out_pos + cnt] = eslot[sl].astype(
                            np.float32
                        )
                        uvals[c, out_pos : out_pos + cnt] = ue[sl].astype(np.float32)
                    out_pos += nt * 128
        assert out_pos == T * 128

    return u, n_tiles, sched, T, idx16, slots, uvals, perm


def _raw_gather(gp, mybir, out_ap, in_ap, idxs_ap, num_idxs, elem_size, elem_step):
    """dma_gather (non-transpose, HBM source) without the 256B-multiple
    elem restriction: elem_size may be any size as long as the source row
    STRIDE (elem_step) is a 256B multiple. Mirrors bass.BassGpSimd.dma_gather."""
    import concourse.ap_utils as ap_utils

    assert idxs_ap.dtype == mybir.dt.int16
    assert in_ap.dtype == out_ap.dtype
    stride_bytes = elem_step * mybir.dt.size(in_ap.dtype)
    assert stride_bytes % 256 == 0 and stride_bytes // 256 < 256
    assert ap_utils.ap_is_contiguous(in_ap.ap[1:])
    assert ap_utils.ap_is_contiguous(out_ap.ap[1:])
    assert ap_utils.ap_is_contiguous(idxs_ap.ap[1:])
    assert in_ap.ap[-1][1] == out_ap.ap[-1][1] == elem_size
    assert in_ap.ap[0][0] == elem_step
    _in_ap = gp.lower_ap_dma(in_ap, for_custom_bir_dma=True)
    _idxs_ap = gp.lower_ap(idxs_ap)
    _out_ap = gp.lower_ap(out_ap)
    return gp.add_instruction(
        mybir.InstDMAGatherAnt(
            name=gp.bass.get_next_instruction_name(),
            ins=[*_in_ap, _idxs_ap, gp.lower_val_access(gp.to_reg(num_idxs))],
            outs=[_out_ap],
            transpose=False,
            num_idxs=num_idxs,
            elem_size=elem_size,
            stride_bytes_256=stride_bytes // 256,
            gen_mode=0,
            single_packet=False,
            queue_num=0,
            sbuf_tokens_per_rank=0,
            sbuf_free_dim_per_rank=0,
            sbuf_free_dim_pad_per_rank=0,
            sbuf_byte_offset=0,
        )
    )


def _build_program(T, sched):
    import concourse.bacc as bacc
    import concourse.mybir as mybir
    from concourse import tile

    f32 = mybir.dt.float32
    bf16 = mybir.dt.bfloat16
    fp8 = mybir.dt.float8e4

    nc = bacc.Bacc(
        "TRN2",
        target_bir_lowering=False,
        debug=False,
        enable_asserts=True,
        num_devices=N_CORES,
    )

    y_d = nc.dram_tensor("y8", [NPAD, YSTRIDE], fp8, kind="ExternalInput").ap()
    idx_d = nc.dram_tensor("idx16", [128, T * 8], mybir.dt.int16, kind="ExternalInput").ap()
    slots_d = nc.dram_tensor("slots", [128, T], f32, kind="ExternalInput").ap()
    uvals_d = nc.dram_tensor("uvals", [128, T], f32, kind="ExternalInput").ap()
    iota_d = nc.dram_tensor("iota", [128, 128], bf16, kind="ExternalInput").ap()
    ident_d = nc.dram_tensor("ident", [128, 128], bf16, kind="ExternalInput").ap()
    ysnm_d = nc.dram_tensor("ysnm", [128, WN * F], bf16, kind="ExternalInput").ap()
    xsnm_d = nc.dram_tensor("xsnm", [128, WN * F], bf16, kind="ExternalInput").ap()
    w_d = nc.dram_tensor("W", [F, F], f32, kind="ExternalInput").ap()
    out_d = nc.dram_tensor("outT", [128, S], bf16, kind="ExternalOutput").ap()

    # last chunk with tiles, per window (for matmul stop flags); -1 = none
    last_ch = {}
    for g, segs in sched:
        for ch in range(N_CHUNKS):
            for w, nt in segs[ch]:
                last_ch[w] = ch

    with tile.TileContext(nc) as tc:
        with (
            tc.tile_pool(name="const", bufs=1) as const_p,
            tc.tile_pool(name="acc", bufs=1) as acc_p,
            tc.tile_pool(name="msgs", bufs=5) as msgs_p,
            tc.tile_pool(name="oh", bufs=4) as oh_p,
            tc.tile_pool(name="psum", bufs=8, space="PSUM") as psum_p,
            tc.tile_pool(name="fin", bufs=3) as fin_p,
        ):
            idx_sb = const_p.tile([128, T * 8], mybir.dt.int16)
            slots_sb = const_p.tile([128, T], f32)
            uvals_sb = const_p.tile([128, T], f32)
            iota_sb = const_p.tile([128, 128], bf16)
            ident_sb = const_p.tile([128, 128], bf16)
            ysnm_sb = const_p.tile([128, WN * F], bf16)
            xsnm_sb = const_p.tile([128, WN * F], bf16)
            w_sb = const_p.tile([F, F], f32)
            acc = acc_p.tile([128, S], f32)

            # consts needed by group-0 compute load first; xsnm/W (tail-only)
            # are deferred into the loop so they don't delay the first gathers
            nc.sync.dma_start(idx_sb[:], idx_d[:])
            nc.sync.dma_start(slots_sb[:], slots_d[:])
            nc.sync.dma_start(uvals_sb[:], uvals_d[:])
            nc.sync.dma_start(iota_sb[:], iota_d[:])
            nc.sync.dma_start(ident_sb[:], ident_d[:])
            nc.sync.dma_start(ysnm_sb[:], ysnm_d[:])

            SL = 512

            def emit_tail(s0, n):
                # out^T chunk = W^T @ acc + xs (xs seeded via identity)
                pf = psum_p.tile([128, SL], f32, tag="psum", name=f"pf_{s0}")
                for j in range((n + 127) // 128):
                    w = s0 // 128 + j
                    nc.tensor.matmul(
                        pf[:, j * 128 : j * 128 + 128],
                        lhsT=xsnm_sb[:, w * F : (w + 1) * F],
                        rhs=ident_sb[:],
                        start=(j == 0),
                        stop=False,
                    )
                nc.tensor.matmul(pf[:, :n], lhsT=w_sb[:], rhs=acc[:, s0 : s0 + n],
                                 start=False, stop=True)
                ot = fin_p.tile([128, SL], bf16, tag="ot")
                nc.scalar.copy(out=ot[:, :n], in_=pf[:, :n])
                nc.sync.dma_start(out_d[:, s0 : s0 + n], ot[:, :n])

            g_tile = 0   # global tile cursor
            tail_s0 = 0  # next output chunk to emit
            for g, segs in sched:
                ws = list(range(g * WG, min((g + 1) * WG, WN)))
                nbank = (len(ws) + 3) // 4
                banks = [psum_p.tile([128, 512], f32, tag="psum", name=f"ps_g{g}_{b}")
                         for b in range(nbank)]

                def wslice(w):
                    wl = w - g * WG
                    return banks[wl // 4][:, (wl % 4) * 128 : (wl % 4) * 128 + 128]

                # seed each window's PSUM quarter with ys (self-loop term).
                # start=True zeroes the whole 2KB bank, so only the FIRST
                # matmul touching a bank may set it; later seeds add onto
                # the zeroed bank with start=False.
                for w in ws:
                    wl = w - g * WG
                    nc.tensor.matmul(
                        wslice(w),
                        lhsT=ysnm_sb[:, w * F : (w + 1) * F],
                        rhs=ident_sb[:],
                        start=(wl % 4 == 0),
                        stop=w not in last_ch,
                    )
                for ch in range(N_CHUNKS):
                    seg_tiles = sum(nt for (_, nt) in segs[ch])
                    if seg_tiles == 0:
                        continue
                    flat = [(wseg, k, nt) for wseg, nt in segs[ch]
                            for k in range(nt)]
                    # split into bounded sub-gathers so msgs buffers stay
                    # small enough to multi-buffer across groups
                    GMAX = 48
                    for sub0 in range(0, seg_tiles, GMAX):
                        sub = flat[sub0 : sub0 + GMAX]
                        ntile = len(sub)
                        n_idx = ntile * 128
                        base = g_tile + sub0
                        msgs = msgs_p.tile([128, ntile * 128], fp8, tag="msgs")
                        m3 = msgs[:].rearrange("p (b f) -> p b f", f=F)
                        _raw_gather(
                            nc.gpsimd, mybir, m3,
                            y_d[ch * CHUNK : (ch + 1) * CHUNK, 0:F],
                            idx_sb[:, base * 8 : base * 8 + n_idx // 16],
                            n_idx, F, YSTRIDE,
                        )
                        # one-hots in 8-tile super-tiles so the tile framework
                        # batches buffer-reuse waits (PE consumes in-order)
                        OHB = 8
                        oh_sup = None
                        for ti, (wseg, k, nt) in enumerate(sub):
                            ob = ti % OHB
                            if ob == 0:
                                oh_sup = oh_p.tile([128, OHB * 128], bf16)
                            gt = base + ti
                            # oh[e, j] = (iota_j == slot_e) * u[dst_e]
                            nc.vector.tensor_scalar(
                                oh_sup[:, ob * 128 : ob * 128 + 128],
                                iota_sb[:],
                                slots_sb[:, gt : gt + 1],
                                uvals_sb[:, gt : gt + 1],
                                mybir.AluOpType.is_equal,
                                mybir.AluOpType.mult,
                            )
                            nc.tensor.matmul(
                                wslice(wseg),
                                lhsT=msgs[:, ti * 128 : (ti + 1) * 128],
                                rhs=oh_sup[:, ob * 128 : ob * 128 + 128],
                                start=False,
                                stop=(last_ch[wseg] == ch and k == nt - 1),
                            )
                    g_tile += seg_tiles
                if g == 0:
                    # tail-only consts: queued behind group 0's gathers so
                    # they don't delay the pipeline start, but emitted before
                    # the first tail chunk reads them
                    nc.sync.dma_start(xsnm_sb[:], xsnm_d[:])
                    nc.sync.dma_start(w_sb[:], w_d[:])
                # flush: Act copies PSUM banks into the (write-once) acc
                for b in range(nbank):
                    c0 = (g * WG + b * 4) * 128
                    ncols = min(512, S - c0)
                    nc.scalar.copy(out=acc[:, c0 : c0 + ncols], in_=banks[b][:, :ncols])
                # emit output chunks whose acc columns are fully flushed
                flushed = min((g + 1) * WG, WN) * 128
                while tail_s0 < S and tail_s0 + min(SL, S - tail_s0) <= flushed:
                    n = min(SL, S - tail_s0)
                    emit_tail(tail_s0, n)
                    tail_s0 += n
            assert g_tile == T
            assert tail_s0 == S

    nc.compile()
    return nc


_PROGRAM_CACHE = {}


def _get_program(T, sched):
    key = (T, tuple((g, tuple(tuple(seg) for seg in segs)) for g, segs in sched))
    if key not in _PROGRAM_CACHE:
        _PROGRAM_CACHE[key] = _build_program(T, sched)
    return _PROGRAM_CACHE[key]


def _prepare(x, edge_index, W, b):
    x = np.asarray(x, dtype=np.float32)
    edge_index = np.asarray(edge_index)
    W = np.asarray(W, dtype=np.float32)
    b = np.asarray(b, dtype=np.float32)

    u, n_tiles, sched, T, idx16, slots, uvals, perm = _host_plan(edge_index)

    import ml_dtypes
    bf = ml_dtypes.bfloat16
    f8 = ml_dtypes.float8_e4m3
    y8 = np.zeros((NPAD, YSTRIDE), dtype=f8)
    y8[:N_NODES, :F] = (u[:, None] * x).astype(f8)

    iota = np.tile(np.arange(128, dtype=np.float32), (128, 1)).astype(bf)
    ident = np.eye(128, dtype=np.float32).astype(bf)

    u_ext = np.concatenate([u, [0.0]]).astype(np.float32)
    x_ext = np.concatenate([x, np.zeros((1, F), np.float32)], axis=0)
    ys_ext = u_ext[:, None] ** 2 * x_ext
    xs_ext = x_ext + b[None, :]

    in_maps = []
    for c in range(N_CORES):
        rows = perm[c]
        idx_c = np.tile(idx16[c].reshape(-1, 16).T, (8, 1)).copy()  # [128, T*8]
        slots_c = slots[c].reshape(T, 128).T.copy()
        uvals_c = uvals[c].reshape(T, 128).T.copy()
        # node-major per-window tiles: [slot-partition, window, feature]
        ysnm = ys_ext[rows].astype(bf).reshape(WN, 128, F).transpose(1, 0, 2)
        xsnm = xs_ext[rows].astype(bf).reshape(WN, 128, F).transpose(1, 0, 2)
        in_maps.append(
            {
                "y8": y8,
                "idx16": idx_c,
                "slots": slots_c.astype(np.float32),
                "uvals": uvals_c.astype(np.float32),
                "iota": iota,
                "ident": ident,
                "ysnm": np.ascontiguousarray(ysnm.reshape(128, WN * F)),
                "xsnm": np.ascontiguousarray(xsnm.reshape(128, WN * F)),
                "W": W,
            }
        )

    nc = _get_program(T, sched)
    global _LAST_PERM
    _LAST_PERM = perm
    return nc, in_maps


_LAST_PERM = None


def _unshard(results, perm=None):
    if perm is None:
        perm = _LAST_PERM
    out = np.empty((N_NODES, F), dtype=np.float32)
    for c in range(N_CORES):
        rows = perm[c]
        valid = rows >= 0
        out[rows[valid]] = results[c]["outT"].T.astype(np.float32)[valid]
    return out


def kernel(x, edge_index, W, b):
    from concourse.bass_utils import run_bass_kernel_spmd

    nc, in_maps = _prepare(x, edge_index, W, b)
    res = run_bass_kernel_spmd(nc, in_maps, list(range(N_CORES)))
    return _unshard(res.results)


if __name__ == "__main__":
    rng = np.random.default_rng(0)
    x = rng.standard_normal((N_NODES, F), dtype=np.float32)
    ei = rng.integers(0, N_NODES, size=(2, 1600000)).astype(np.int64)
    W = rng.standard_normal((F, F), dtype=np.float32) / np.sqrt(F)
    b = np.zeros(F, dtype=np.float32)
    out = kernel(x=x, edge_index=ei, W=W, b=b)
    print(out.shape, out.dtype)


# revision 24
# speedup vs baseline: 1.8497x; 1.0252x over previous
"""GCNConvSC (residual + GCNConv) Trainium2 Bass kernel, 8-core SPMD.

Math (matches the PyG-style reference):
    deg[v]  = indeg_with_selfloop(v)          (count of v in dst, +1)
    u       = deg^{-1/2}
    y       = u[:,None] * x                   (pre-scaled node features, fp8)
    z[v]    = sum_{e: dst_e = v} y[src_e] * u[v]   (via one-hot matmuls)
    out[v]  = x[v] + b + (z[v] + u[v]^2 * x[v]) @ W

Pipeline per core (dst nodes range-partitioned, S=12544 slots, 98 windows
of 128):
  - y stored in HBM as fp8 e4m3 rows padded to a 256B stride; per-edge rows
    are fetched with a raw InstDMAGatherAnt (elem_size=128, elem_step=256),
    i.e. 128B descriptors, which the DMA cost model prices at half the
    256B-descriptor rate.  Edges are bucketed by (window-group, src-chunk,
    window) with int16 chunk-local indices (4 chunks of 25024 rows).
  - Aggregation: per 128-edge tile a bf16 one-hot (iota==slot)*u[dst] is
    built on DVE (4x perf mode) and matmul'd (fp8 lhsT x bf16 rhs) into a
    PSUM bank quarter for the edge's dst window.
  - The self-loop term ys = u^2*x and the residual xs = x + b are seeded
    into PSUM by identity-rhs matmuls (lhsT = node-major bf16 tiles), so
    the SBUF accumulator is write-once and flushes are plain Activation-
    engine PSUM->SBUF copies (DVE stays free for one-hots).
  - Tail: out^T = W^T @ acc accumulated on top of the xs seed, copied to
    bf16 and stored.
"""

import sys

sys.path.insert(0, "/opt/trn_rl_repo")

import numpy as np

N_NODES = 100000
F = 128
N_CORES = 8
S = 12544            # dst slots per core (98 windows of 128)
WN = 98              # windows per core
WG = 16              # windows per PSUM group (4 banks of 4 windows)
N_CHUNKS = 4
CHUNK = 25024        # gather-source rows per chunk (int16-safe)
NPAD = N_CHUNKS * CHUNK  # 100096 padded node rows for y
YSTRIDE = 256        # fp8 row stride in bytes (DMA desc stride granularity)


def _host_plan(edge_index):
    """Sort/bucket edges per core; emit the shared SPMD schedule plus
    per-core gather-index and slot arrays."""
    src = np.asarray(edge_index[0], dtype=np.int64)
    dst = np.asarray(edge_index[1], dtype=np.int64)

    deg_e = np.bincount(dst, minlength=N_NODES)
    u = (1.0 / np.sqrt(deg_e.astype(np.float64) + 1.0)).astype(np.float32)

    # Deal dsts snake-wise by descending degree across cores: every core's
    # position-p dst has ~the same degree, so per-(chunk, window) counts are
    # nearly equal across cores and the shared max-based schedule pads little.
    order = np.argsort(-deg_e, kind="stable")
    i = np.arange(N_NODES)
    blk, lane = i // N_CORES, i % N_CORES
    core_i = np.where(blk % 2 == 0, lane, N_CORES - 1 - lane)
    perm = np.full((N_CORES, S), -1, dtype=np.int64)
    perm[core_i, blk] = order
    core_of_node = np.empty(N_NODES, dtype=np.int64)
    pos_of_node = np.empty(N_NODES, dtype=np.int64)
    core_of_node[order] = core_i
    pos_of_node[order] = blk

    core_of = core_of_node[dst]
    pos_e_all = pos_of_node[dst]
    u_e_all = u[dst]
    chunk_of = src // CHUNK

    per_core = []
    counts = np.zeros((N_CORES, N_CHUNKS, WN), dtype=np.int64)
    for c in range(N_CORES):
        m = core_of == c
        es, pos_e, ue = src[m], pos_e_all[m], u_e_all[m]
        ch = chunk_of[m]
        w = pos_e // 128
        slot = pos_e % 128
        wg = w // WG
        so = np.lexsort((w, ch, wg))
        es, slot, ch, w, ue = es[so], slot[so], ch[so], w[so], ue[so]
        np.add.at(counts[c], (ch, w), 1)
        per_core.append((es, slot, ch, w, ue))

    # shared schedule: tiles per (chunk, window) = max over cores; windows
    # with zero edges need no tiles (their PSUM quarter is seeded anyway)
    n_tiles = (counts.max(axis=0) + 127) // 128  # [N_CHUNKS, WN]

    n_wg = (WN + WG - 1) // WG
    sched = []  # (g, segs) with segs[ch] = [(window, ntiles), ...]
    T = 0
    for g in range(n_wg):
        ws = range(g * WG, min((g + 1) * WG, WN))
        segs = []
        for ch in range(N_CHUNKS):
            tl = [(w, int(n_tiles[ch, w])) for w in ws if n_tiles[ch, w] > 0]
            segs.append(tl)
        sched.append((g, segs))
        T += int(n_tiles[:, list(ws)].sum())

    # per-core padded edge streams in schedule order
    idx16 = np.zeros((N_CORES, T * 128), dtype=np.int16)
    slots = np.full((N_CORES, T * 128), -1.0, dtype=np.float32)
    uvals = np.zeros((N_CORES, T * 128), dtype=np.float32)
    for c in range(N_CORES):
        es, eslot, ch, w, ue = per_core[c]
        keys = list(zip(w // WG, ch, w))
        run_start = {}
        for i2, k in enumerate(keys):
            if k not in run_start:
                run_start[k] = i2
        run_len = counts[c]
        out_pos = 0
        for g, segs in sched:
            for chp in range(N_CHUNKS):
                for wseg, nt in segs[chp]:
                    cnt = int(run_len[chp, wseg])
                    if cnt > 0:
                        i0 = run_start[(g, chp, wseg)]
                        sl = slice(i0, i0 + cnt)
                        local = (es[sl] - chp * CHUNK).astype(np.int16)
                        idx16[c, out_pos : out_pos + cnt] = local
                        slots[c, out_pos : out_pos + cnt] = eslot[sl].astype(
                            np.float32
                        )
                        uvals[c, out_pos : out_pos + cnt] = ue[sl].astype(np.float32)
                    out_pos += nt * 128
        assert out_pos == T * 128

    return u, n_tiles, sched, T, idx16, slots, uvals, perm


def _raw_gather(gp, mybir, out_ap, in_ap, idxs_ap, num_idxs, elem_size, elem_step):
    """dma_gather (non-transpose, HBM source) without the 256B-multiple
    elem restriction: elem_size may be any size as long as the source row
    STRIDE (elem_step) is a 256B multiple. Mirrors bass.BassGpSimd.dma_gather."""
    import concourse.ap_utils as ap_utils

    assert idxs_ap.dtype == mybir.dt.int16
    assert in_ap.dtype == out_ap.dtype
    stride_bytes = elem_step * mybir.dt.size(in_ap.dtype)
    assert stride_bytes % 256 == 0 and stride_bytes // 256 < 256
    assert ap_utils.ap_is_contiguous(in_ap.ap[1:])
    assert ap_utils.ap_is_contiguous(out_ap.ap[1:])
    assert ap_utils.ap_is_contiguous(idxs_ap.ap[1:])
    assert in_ap.ap[-1][1] == out_ap.ap[-1][1] == elem_size
    assert in_ap.ap[0][0] == elem_step
    _in_ap = gp.lower_ap_dma(in_ap, for_custom_bir_dma=True)
    _idxs_ap = gp.lower_ap(idxs_ap)
    _out_ap = gp.lower_ap(out_ap)
    return gp.add_instruction(
        mybir.InstDMAGatherAnt(
            name=gp.bass.get_next_instruction_name(),
            ins=[*_in_ap, _idxs_ap, gp.lower_val_access(gp.to_reg(num_idxs))],
            outs=[_out_ap],
            transpose=False,
            num_idxs=num_idxs,
            elem_size=elem_size,
            stride_bytes_256=stride_bytes // 256,
            gen_mode=0,
            single_packet=False,
            queue_num=0,
            sbuf_tokens_per_rank=0,
            sbuf_free_dim_per_rank=0,
            sbuf_free_dim_pad_per_rank=0,
            sbuf_byte_offset=0,
        )
    )


def _build_program(T, sched, has_bias):
    import concourse.bacc as bacc
    import concourse.mybir as mybir
    from concourse import tile

    f32 = mybir.dt.float32
    bf16 = mybir.dt.bfloat16
    fp8 = mybir.dt.float8e4

    nc = bacc.Bacc(
        "TRN2",
        target_bir_lowering=False,
        debug=False,
        enable_asserts=True,
        num_devices=N_CORES,
    )

    y_d = nc.dram_tensor("y8", [NPAD, YSTRIDE], fp8, kind="ExternalInput").ap()
    idx_d = nc.dram_tensor("idx16", [128, T * 8], mybir.dt.int16, kind="ExternalInput").ap()
    slots_d = nc.dram_tensor("slots", [128, T], f32, kind="ExternalInput").ap()
    uvals_d = nc.dram_tensor("uvals", [128, T], f32, kind="ExternalInput").ap()
    iota_d = nc.dram_tensor("iota", [128, 128], bf16, kind="ExternalInput").ap()
    ident_d = nc.dram_tensor("ident", [128, 128], bf16, kind="ExternalInput").ap()
    pidx_d = nc.dram_tensor("pidx", [128, 1], f32, kind="ExternalInput").ap()
    u2w_d = nc.dram_tensor("u2w", [128, WN], f32, kind="ExternalInput").ap()
    xsnm_d = nc.dram_tensor("xsnm", [128, WN * F], bf16, kind="ExternalInput").ap()
    # with a nonzero bias the self-loop seed needs plain x (not x+b)
    xnm_d = (nc.dram_tensor("xnm", [128, WN * F], bf16, kind="ExternalInput").ap()
             if has_bias else xsnm_d)
    w_d = nc.dram_tensor("W", [F, F], f32, kind="ExternalInput").ap()
    out_d = nc.dram_tensor("outT", [128, S], bf16, kind="ExternalOutput").ap()

    # last chunk with tiles, per window (for matmul stop flags); -1 = none
    last_ch = {}
    for g, segs in sched:
        for ch in range(N_CHUNKS):
            for w, nt in segs[ch]:
                last_ch[w] = ch

    with tile.TileContext(nc) as tc:
        with (
            tc.tile_pool(name="const", bufs=1) as const_p,
            tc.tile_pool(name="acc", bufs=1) as acc_p,
            tc.tile_pool(name="msgs", bufs=5) as msgs_p,
            tc.tile_pool(name="oh", bufs=4) as oh_p,
            tc.tile_pool(name="diag", bufs=4) as diag_p,
            tc.tile_pool(name="psum", bufs=8, space="PSUM") as psum_p,
            tc.tile_pool(name="fin", bufs=3) as fin_p,
        ):
            idx_sb = const_p.tile([128, T * 8], mybir.dt.int16)
            slots_sb = const_p.tile([128, T], f32)
            uvals_sb = const_p.tile([128, T], f32)
            iota_sb = const_p.tile([128, 128], bf16)
            ident_sb = const_p.tile([128, 128], bf16)
            pidx_sb = const_p.tile([128, 1], f32)
            u2w_sb = const_p.tile([128, WN], f32)
            xsnm_sb = const_p.tile([128, WN * F], bf16)
            xnm_sb = (const_p.tile([128, WN * F], bf16) if has_bias else xsnm_sb)
            w_sb = const_p.tile([F, F], f32)
            acc = acc_p.tile([128, S], f32)

            # consts needed by group-0 compute load first; W (tail-only)
            # is deferred into the loop so it doesn't delay the first gathers
            nc.sync.dma_start(idx_sb[:], idx_d[:])
            nc.sync.dma_start(slots_sb[:], slots_d[:])
            nc.sync.dma_start(uvals_sb[:], uvals_d[:])
            nc.sync.dma_start(iota_sb[:], iota_d[:])
            nc.sync.dma_start(ident_sb[:], ident_d[:])
            nc.sync.dma_start(pidx_sb[:], pidx_d[:])
            nc.sync.dma_start(u2w_sb[:], u2w_d[:])
            nc.sync.dma_start(xsnm_sb[:], xsnm_d[:])
            if has_bias:
                nc.sync.dma_start(xnm_sb[:], xnm_d[:])

            SL = 512

            def emit_tail(s0, n):
                # out^T chunk = W^T @ acc + xs (xs seeded via identity)
                pf = psum_p.tile([128, SL], f32, tag="psum", name=f"pf_{s0}")
                for j in range((n + 127) // 128):
                    w = s0 // 128 + j
                    nc.tensor.matmul(
                        pf[:, j * 128 : j * 128 + 128],
                        lhsT=xsnm_sb[:, w * F : (w + 1) * F],
                        rhs=ident_sb[:],
                        start=(j == 0),
                        stop=False,
                    )
                nc.tensor.matmul(pf[:, :n], lhsT=w_sb[:], rhs=acc[:, s0 : s0 + n],
                                 start=False, stop=True)
                ot = fin_p.tile([128, SL], bf16, tag="ot")
                nc.scalar.copy(out=ot[:, :n], in_=pf[:, :n])
                nc.sync.dma_start(out_d[:, s0 : s0 + n], ot[:, :n])

            g_tile = 0   # global tile cursor
            tail_s0 = 0  # next output chunk to emit
            for g, segs in sched:
                ws = list(range(g * WG, min((g + 1) * WG, WN)))
                nbank = (len(ws) + 3) // 4
                banks = [psum_p.tile([128, 512], f32, tag="psum", name=f"ps_g{g}_{b}")
                         for b in range(nbank)]

                def wslice(w):
                    wl = w - g * WG
                    return banks[wl // 4][:, (wl % 4) * 128 : (wl % 4) * 128 + 128]

                # seed each window's PSUM quarter with the self-loop term
                # u^2*x via a diagonal rhs: diag[p, j] = (j == p) * u^2_p.
                # start=True zeroes the whole 2KB bank, so only the FIRST
                # matmul touching a bank may set it; later seeds add onto
                # the zeroed bank with start=False.
                for w in ws:
                    wl = w - g * WG
                    dg = diag_p.tile([128, 128], bf16, tag="diag")
                    nc.vector.tensor_scalar(
                        dg[:],
                        iota_sb[:],
                        pidx_sb[:, 0:1],
                        u2w_sb[:, w : w + 1],
                        mybir.AluOpType.is_equal,
                        mybir.AluOpType.mult,
                    )
                    nc.tensor.matmul(
                        wslice(w),
                        lhsT=xnm_sb[:, w * F : (w + 1) * F],
                        rhs=dg[:],
                        start=(wl % 4 == 0),
                        stop=w not in last_ch,
                    )
                for ch in range(N_CHUNKS):
                    seg_tiles = sum(nt for (_, nt) in segs[ch])
                    if seg_tiles == 0:
                        continue
                    flat = [(wseg, k, nt) for wseg, nt in segs[ch]
                            for k in range(nt)]
                    # split into bounded sub-gathers so msgs buffers stay
                    # small enough to multi-buffer across groups
                    GMAX = 48
                    for sub0 in range(0, seg_tiles, GMAX):
                        sub = flat[sub0 : sub0 + GMAX]
                        ntile = len(sub)
                        n_idx = ntile * 128
                        base = g_tile + sub0
                        msgs = msgs_p.tile([128, ntile * 128], fp8, tag="msgs")
                        m3 = msgs[:].rearrange("p (b f) -> p b f", f=F)
                        _raw_gather(
                            nc.gpsimd, mybir, m3,
                            y_d[ch * CHUNK : (ch + 1) * CHUNK, 0:F],
                            idx_sb[:, base * 8 : base * 8 + n_idx // 16],
                            n_idx, F, YSTRIDE,
                        )
                        # one-hots in 8-tile super-tiles so the tile framework
                        # batches buffer-reuse waits (PE consumes in-order)
                        OHB = 8
                        oh_sup = None
                        for ti, (wseg, k, nt) in enumerate(sub):
                            ob = ti % OHB
                            if ob == 0:
                                oh_sup = oh_p.tile([128, OHB * 128], bf16)
                            gt = base + ti
                            # oh[e, j] = (iota_j == slot_e) * u[dst_e]
                            nc.vector.tensor_scalar(
                                oh_sup[:, ob * 128 : ob * 128 + 128],
                                iota_sb[:],
                                slots_sb[:, gt : gt + 1],
                                uvals_sb[:, gt : gt + 1],
                                mybir.AluOpType.is_equal,
                                mybir.AluOpType.mult,
                            )
                            nc.tensor.matmul(
                                wslice(wseg),
                                lhsT=msgs[:, ti * 128 : (ti + 1) * 128],
                                rhs=oh_sup[:, ob * 128 : ob * 128 + 128],
                                start=False,
                                stop=(last_ch[wseg] == ch and k == nt - 1),
                            )
                    g_tile += seg_tiles
                if g == 0:
                    # tail-only const: queued behind group 0's gathers so it
                    # doesn't delay the pipeline start, but emitted before
                    # the first tail chunk reads it
                    nc.sync.dma_start(w_sb[:], w_d[:])
                # flush: Act copies PSUM banks into the (write-once) acc
                for b in range(nbank):
                    c0 = (g * WG + b * 4) * 128
                    ncols = min(512, S - c0)
                    nc.scalar.copy(out=acc[:, c0 : c0 + ncols], in_=banks[b][:, :ncols])
                # emit output chunks whose acc columns are fully flushed
                flushed = min((g + 1) * WG, WN) * 128
                while tail_s0 < S and tail_s0 + min(SL, S - tail_s0) <= flushed:
                    n = min(SL, S - tail_s0)
                    emit_tail(tail_s0, n)
                    tail_s0 += n
            assert g_tile == T
            assert tail_s0 == S

    nc.compile()
    return nc


_PROGRAM_CACHE = {}


def _get_program(T, sched, has_bias):
    key = (T, has_bias,
           tuple((g, tuple(tuple(seg) for seg in segs)) for g, segs in sched))
    if key not in _PROGRAM_CACHE:
        _PROGRAM_CACHE[key] = _build_program(T, sched, has_bias)
    return _PROGRAM_CACHE[key]


def _prepare(x, edge_index, W, b):
    x = np.asarray(x, dtype=np.float32)
    edge_index = np.asarray(edge_index)
    W = np.asarray(W, dtype=np.float32)
    b = np.asarray(b, dtype=np.float32)

    u, n_tiles, sched, T, idx16, slots, uvals, perm = _host_plan(edge_index)

    import ml_dtypes
    bf = ml_dtypes.bfloat16
    f8 = ml_dtypes.float8_e4m3
    y8 = np.zeros((NPAD, YSTRIDE), dtype=f8)
    y8[:N_NODES, :F] = (u[:, None] * x).astype(f8)

    iota = np.tile(np.arange(128, dtype=np.float32), (128, 1)).astype(bf)
    ident = np.eye(128, dtype=np.float32).astype(bf)

    u_ext = np.concatenate([u, [0.0]]).astype(np.float32)
    x_ext = np.concatenate([x, np.zeros((1, F), np.float32)], axis=0)
    xs_ext = x_ext + b[None, :]
    has_bias = bool(np.any(b != 0))
    pidx = np.arange(128, dtype=np.float32).reshape(128, 1)

    in_maps = []
    for c in range(N_CORES):
        rows = perm[c]
        idx_c = np.tile(idx16[c].reshape(-1, 16).T, (8, 1)).copy()  # [128, T*8]
        slots_c = slots[c].reshape(T, 128).T.copy()
        uvals_c = uvals[c].reshape(T, 128).T.copy()
        # node-major per-window tiles: [slot-partition, window, feature]
        xsnm = xs_ext[rows].astype(bf).reshape(WN, 128, F).transpose(1, 0, 2)
        u2w = (u_ext[rows] ** 2).astype(np.float32).reshape(WN, 128).T
        im = {
            "y8": y8,
            "idx16": idx_c,
            "slots": slots_c.astype(np.float32),
            "uvals": uvals_c.astype(np.float32),
            "iota": iota,
            "ident": ident,
            "pidx": pidx,
            "u2w": np.ascontiguousarray(u2w),
            "xsnm": np.ascontiguousarray(xsnm.reshape(128, WN * F)),
            "W": W,
        }
        if has_bias:
            xnm = x_ext[rows].astype(bf).reshape(WN, 128, F).transpose(1, 0, 2)
            im["xnm"] = np.ascontiguousarray(xnm.reshape(128, WN * F))
        in_maps.append(im)

    nc = _get_program(T, sched, has_bias)
    global _LAST_PERM
    _LAST_PERM = perm
    return nc, in_maps


_LAST_PERM = None


def _unshard(results, perm=None):
    if perm is None:
        perm = _LAST_PERM
    out = np.empty((N_NODES, F), dtype=np.float32)
    for c in range(N_CORES):
        rows = perm[c]
        valid = rows >= 0
        out[rows[valid]] = results[c]["outT"].T.astype(np.float32)[valid]
    return out


def kernel(x, edge_index, W, b):
    from concourse.bass_utils import run_bass_kernel_spmd

    nc, in_maps = _prepare(x, edge_index, W, b)
    res = run_bass_kernel_spmd(nc, in_maps, list(range(N_CORES)))
    return _unshard(res.results)


if __name__ == "__main__":
    rng = np.random.default_rng(0)
    x = rng.standard_normal((N_NODES, F), dtype=np.float32)
    ei = rng.integers(0, N_NODES, size=(2, 1600000)).astype(np.int64)
    W = rng.standard_normal((F, F), dtype=np.float32) / np.sqrt(F)
    b = np.zeros(F, dtype=np.float32)
    out = kernel(x=x, edge_index=ei, W=W, b=b)
    print(out.shape, out.dtype)
